# revision 1
# baseline (speedup 1.0000x reference)
"""KroneckerLinear Trainium2 kernel.

Math: out = x @ kron(f1, f2).T + bias, with x [64, 8192], f1 [128,128],
f2 [64,64], bias [8192].  Kronecker identity:
    out[b].reshape(128, 64) = f1 @ X_b @ f2.T,   X_b = x[b].reshape(128, 64)
so the 8192x8192 weight (256 MB) is never materialized; the kernel is
memory-bound on x in / out (~4 MB total).

Sharding: batch-parallel over the 8 NeuronCores, 8 batch rows per core.

Per-core device program (SPMD, identical on all cores), pipelined in 4
column slices p = 0..3:
  stage 1 (apply f2): matmul p has lhsT = xt[:, p*128:(p+1)*128] where xt
     is a host-prepared [128, 512] tile: rows (h*64+l), cols (p*128+j)
     hold x[lb, j*64+l] for local batch lb = p + 4h.  rhs =
     blkdiag(f2.T, f2.T) [128,128]: one K=128 matmul computes TWO
     batches: out[j, h*64+k] = (X_{p+4h} @ f2.T)[j, k].  Each slice gets
     its own PSUM bank so the stage-2 pipeline never bank-conflicts.
  stage 2 (apply f1): per slice: DVE copy V slice to SBUF, matmul
     lhsT = f1.T, rhs = V slice -> Y slice [i, (g, k)], g in {2p, 2p+1},
     local batch(g) = g//2 + 4*(g%2).
  bias: per slice, Y + bias.reshape(128, 64) broadcast over both groups
     (fused with the PSUM->SBUF move).
  store: per slice, 3-D DMA to y rows (2p, 2p+1); host unpermutes rows.
"""

import numpy as np

N_CORES = 8
B = 64
LB = B // N_CORES  # 8 local batches per core

_CACHE = {}


def _build_nc(use_f32r=False):
    import concourse.bass as bass
    import concourse.mybir as mybir
    import concourse.tile as tile
    from concourse import bacc

    fp32 = mybir.dt.float32
    mmdt = mybir.dt.float32r if use_f32r else fp32

    nc = bacc.Bacc("TRN2", target_bir_lowering=False, debug=False)
    # all inputs packed into one [128, 832] tensor:
    # blk 0:128 | f1t 128:256 | xt_p0 256:384 | biasr 384:448 | xt_p123 448:832
    in_d = nc.dram_tensor("inp", [128, 832], fp32, kind="ExternalInput")
    y_d = nc.dram_tensor("y", [LB, 8192], fp32, kind="ExternalOutput")

    with tile.TileContext(nc) as tc:
        with (
            tc.tile_pool(name="sb", bufs=1) as sb,
            tc.tile_pool(name="slc", bufs=4) as slc,
            tc.tile_pool(name="osb", bufs=2) as osb,
            tc.tile_pool(name="psv", bufs=4, space="PSUM") as psv,
            tc.tile_pool(name="psy", bufs=4, space="PSUM") as psy,
        ):
            inp = sb.tile([128, 832], fp32)
            blk = inp[:, 0:128]
            f1t = inp[:, 128:256]
            biasr = inp[:, 384:448]

            def xt_slice(p):
                return inp[:, 256:384] if p == 0 else inp[:, 320 + p * 128 : 448 + p * 128]

            # DMA 1: everything slice-0 compute needs; DMA 2: the rest.
            nc.sync.dma_start(out=inp[:, 0:384], in_=in_d[:, 0:384])
            nc.sync.dma_start(out=inp[:, 384:832], in_=in_d[:, 384:832])

            b_ap = biasr
            bias_bcast = bass.AP(
                tensor=b_ap.tensor,
                offset=b_ap.offset,
                ap=[b_ap.ap[0], [0, 2], b_ap.ap[1]],
            )

            def mm(ap):
                return ap.bitcast(mmdt) if use_f32r else ap

            out_halves = []
            for _h in range(2):
                out_half = osb.tile([128, 256], fp32, tag="out_sb")
                out_halves.append(out_half)

            v_all = sb.tile([128, 512], fp32)
            for p in range(4):
                psum_v = psv.tile([128, 128], fp32, tag="psum_v")
                nc.tensor.matmul(
                    psum_v[:], mm(xt_slice(p)), mm(blk), start=True, stop=True
                )
                # V copy on ACT (DVE is the busier engine: it owns the adds)
                nc.scalar.copy(v_all[:, p * 128 : (p + 1) * 128], psum_v[:])

            # stage 2: one N=512 matmul (f32r runs this at full rate)
            psum_y = psy.tile([128, 512], fp32)
            nc.tensor.matmul(psum_y[:], mm(f1t), mm(v_all[:, :]), start=True, stop=True)

            for p in range(4):
                out_sb = out_halves[p // 2]
                o_ap = out_sb[:, (p % 2) * 128 : (p % 2) * 128 + 128]
                out_g = bass.AP(
                    tensor=o_ap.tensor,
                    offset=o_ap.offset,
                    ap=[o_ap.ap[0], [64, 2], [1, 64]],
                )
                y_ap = psum_y[:, p * 128 : (p + 1) * 128]
                y_g = bass.AP(
                    tensor=y_ap.tensor,
                    offset=y_ap.offset,
                    ap=[y_ap.ap[0], [64, 2], [1, 64]],
                )
                nc.vector.tensor_add(out_g, y_g, bias_bcast)

                if p % 2 == 1:
                    # store half -> y rows (2p-2 .. 2p+1) in group order
                    s_ap = out_sb[:, :]
                    src = bass.AP(
                        tensor=s_ap.tensor,
                        offset=s_ap.offset,
                        ap=[s_ap.ap[0], [64, 4], [1, 64]],
                    )
                    d_ap = y_d[:, :]
                    dst = bass.AP(
                        tensor=d_ap.tensor,
                        offset=d_ap.offset + (2 * p - 2) * 8192,
                        ap=[[64, 128], [8192, 4], [1, 64]],
                    )
                    # SP is free after the input loads
                    nc.sync.dma_start(out=dst, in_=src)

    nc.compile()
    return nc


def _prep_core_inputs(x, factor1, factor2, bias):
    """Host-side layout prep. Returns list of per-core in_maps."""
    x = np.ascontiguousarray(np.asarray(x, dtype=np.float32))
    f1 = np.asarray(factor1, dtype=np.float32)
    f2 = np.asarray(factor2, dtype=np.float32)
    bias = np.asarray(bias, dtype=np.float32)

    # x -> per-core xt [128, 512]: xt[h*64+l, p*128+j] = x[c*8 + p + 4h, j*64+l]
    xc = x.reshape(N_CORES, LB, 128, 64)  # [c, lb, j, l]
    arr = xc.transpose(0, 3, 1, 2).reshape(N_CORES, 64, 2, 4, 128)
    xt_all = arr.transpose(0, 2, 1, 3, 4).reshape(N_CORES, 128, 512)

    # packed input [128, 832]: blk | f1t | xt_p0 | biasr | xt_p123
    inp_all = np.zeros((N_CORES, 128, 832), dtype=np.float32)
    inp_all[:, :64, 0:64] = f2.T
    inp_all[:, 64:, 64:128] = f2.T
    inp_all[:, :, 128:256] = f1.T
    inp_all[:, :, 256:384] = xt_all[:, :, 0:128]
    inp_all[:, :, 384:448] = bias.reshape(128, 64)
    inp_all[:, :, 448:832] = xt_all[:, :, 128:512]

    return [{"inp": np.ascontiguousarray(inp_all[c])} for c in range(N_CORES)]


def kernel(x, factor1, factor2, bias):
    from concourse.bass_utils import run_bass_kernel_spmd

    if "nc" not in _CACHE:
        _CACHE["nc"] = _build_nc()
    nc = _CACHE["nc"]

    in_maps = _prep_core_inputs(x, factor1, factor2, bias)
    res = run_bass_kernel_spmd(nc, in_maps, core_ids=list(range(N_CORES)))
    kernel.last_results = res

    # device writes y rows in group order g (batch = g//2 + 4*(g%2));
    # unpermute to batch order: inv = argsort([0,4,1,5,2,6,3,7])
    inv = np.array([0, 2, 4, 6, 1, 3, 5, 7])
    out = np.concatenate(
        [res.results[c]["y"][inv] for c in range(N_CORES)], axis=0
    )
    return out



# revision 6
# speedup vs baseline: 1.1026x; 1.1026x over previous
"""KroneckerLinear Trainium2 kernel.

Math: out = x @ kron(f1, f2).T + bias, with x [64, 8192], f1 [128,128],
f2 [64,64], bias [8192].  Kronecker identity:
    out[b].reshape(128, 64) = f1 @ X_b @ f2.T,   X_b = x[b].reshape(128, 64)
so the 8192x8192 weight (256 MB) is never materialized; the kernel is
memory-bound on x in / out (~4 MB total).

Sharding: batch-parallel over the 8 NeuronCores, 8 batch rows per core.

Per-core device program (SPMD, identical on all cores):
  loads: two parallel DMAs on the two HWDGE rings (sync + scalar), each
     from a fully contiguous DRAM tensor.
     A = blk | f1t | xt0 | xt1  (512 cols), B = xt2 | xt3 | biasr (320).
     xt[h*64+l, p*128+j] = x[lb, j*64+l] for local batch lb = p + 4h;
     blk = blkdiag(f2.T, f2.T) so one K=128 matmul computes TWO batches.
  stage 1 (apply f2): matmul p: lhsT = xt_p, rhs = blk ->
     psum_v[:, p*128:...][j, h*64+k] = (X_{p+4h} @ f2.T)[j, k].
     PSUM->SBUF copies alternate gpsimd/vector (no scalar ACTIVATE =>
     no act-table load contending with the input DMA).
  stage 2 (apply f1): two matmuls lhsT = f1t, rhs = v half [128, 256]
     in f32r (N=256 hits the 1 cycle/row full-rate path; fp32 runs at
     4 cycles/row via two half-speed passes + a PE drain).
  bias: fused with the PSUM->SBUF move; half 0 on vector, half 1 on
     gpsimd, in parallel.  bias.reshape(128, 64) broadcast over the 4
     (p, h) groups of each half.
  stores: two contiguous DMAs (sync + scalar rings in parallel) of the
     device-natural layout y[i, p*128+h*64+k]; host unpermutes.
"""

import numpy as np

N_CORES = 8
B = 64
LB = B // N_CORES  # 8 local batches per core

_CACHE = {}


def _build_nc(f32r_s1=True, f32r_s2=True):
    import concourse.bass as bass
    import concourse.mybir as mybir
    import concourse.tile as tile
    from concourse import bacc

    fp32 = mybir.dt.float32
    f32r = mybir.dt.float32r

    nc = bacc.Bacc("TRN2", target_bir_lowering=False, debug=False)
    # contiguous per-ring inputs (f32r = fp32 bit layout + rounding tag so
    # the BIR verifier accepts them as FP32r matmul operands):
    # A: blk 0:128 | f1t 128:256 | xt0 256:384 | xt1 384:512
    # B: xt2 0:128 | xt3 128:256 | biasr 256:320
    inA_d = nc.dram_tensor("inpA", [128, 512], f32r, kind="ExternalInput")
    inB_d = nc.dram_tensor("inpB", [128, 320], f32r, kind="ExternalInput")
    y0_d = nc.dram_tensor("y0", [128, 256], fp32, kind="ExternalOutput")
    y1_d = nc.dram_tensor("y1", [128, 256], fp32, kind="ExternalOutput")

    def mm1(ap):
        return ap if f32r_s1 else ap.bitcast(fp32)

    def mm2(ap):
        return ap if f32r_s2 else ap.bitcast(fp32)

    with tile.TileContext(nc) as tc:
        with (
            tc.tile_pool(name="sb", bufs=1) as sb,
            tc.tile_pool(name="ps", bufs=2, space="PSUM") as ps,
        ):
            sbA = sb.tile([128, 512], f32r)
            sbB = sb.tile([128, 320], f32r)
            v = sb.tile([128, 512], f32r)
            out0 = sb.tile([128, 256], fp32)
            out1 = sb.tile([128, 256], fp32)

            blk = sbA[:, 0:128]
            f1t = sbA[:, 128:256]
            biasr = sbB[:, 256:320]

            def xt_slice(p):
                return sbA[:, 256 + p * 128 : 384 + p * 128] if p < 2 else sbB[
                    :, (p - 2) * 128 : (p - 1) * 128
                ]

            # two parallel input DMAs, one per HWDGE ring
            nc.sync.dma_start(out=sbA[:, :], in_=inA_d[:, :])
            nc.scalar.dma_start(out=sbB[:, :], in_=inB_d[:, :])

            b_ap = biasr.bitcast(fp32)
            bias_bcast = bass.AP(
                tensor=b_ap.tensor,
                offset=b_ap.offset,
                ap=[b_ap.ap[0], [0, 4], b_ap.ap[1]],
            )

            # stage 1: 4 matmuls into one PSUM bank
            psum_v = ps.tile([128, 512], fp32, tag="psum_v")
            for p in range(4):
                nc.tensor.matmul(
                    psum_v[:, p * 128 : (p + 1) * 128],
                    mm1(xt_slice(p)),
                    mm1(blk),
                    start=True,
                    stop=True,
                )
            # PSUM -> SBUF copies; gpsimd can't read PSUM on TRN2, so
            # alternate scalar (ACTIVATE) / vector (DVE)
            for p in range(4):
                src = psum_v[:, p * 128 : (p + 1) * 128]
                dst = v[:, p * 128 : (p + 1) * 128]
                if p % 2 == 0:
                    nc.scalar.copy(dst, src)
                else:
                    nc.vector.tensor_copy(dst, src)

            # stage 2: two N=256 matmuls (f32r full rate)
            psum_y = []
            for hlf in range(2):
                py = ps.tile([128, 256], fp32, tag=f"psum_y{hlf}")
                psum_y.append(py)
                nc.tensor.matmul(
                    py[:],
                    mm2(f1t),
                    mm2(v[:, hlf * 256 : (hlf + 1) * 256]),
                    start=True,
                    stop=True,
                )

            # bias add fused with PSUM->SBUF move (DVE; gpsimd can't read PSUM)
            for hlf, (out_sb, eng) in enumerate(
                zip([out0, out1], [nc.vector, nc.vector])
            ):
                o_ap = out_sb[:, :]
                out_g = bass.AP(
                    tensor=o_ap.tensor, offset=o_ap.offset,
                    ap=[o_ap.ap[0], [64, 4], [1, 64]],
                )
                y_ap = psum_y[hlf][:, :]
                y_g = bass.AP(
                    tensor=y_ap.tensor, offset=y_ap.offset,
                    ap=[y_ap.ap[0], [64, 4], [1, 64]],
                )
                eng.tensor_add(out_g, y_g, bias_bcast)

            # two parallel contiguous output DMAs
            nc.sync.dma_start(out=y0_d[:, :], in_=out0[:, :])
            nc.scalar.dma_start(out=y1_d[:, :], in_=out1[:, :])

    nc.compile()
    return nc


def _prep_core_inputs(x, factor1, factor2, bias):
    """Host-side layout prep. Returns list of per-core in_maps."""
    x = np.ascontiguousarray(np.asarray(x, dtype=np.float32))
    f1 = np.asarray(factor1, dtype=np.float32)
    f2 = np.asarray(factor2, dtype=np.float32)
    bias = np.asarray(bias, dtype=np.float32)

    # x -> per-core xt [128, 512]: xt[h*64+l, p*128+j] = x[c*8 + p + 4h, j*64+l]
    xc = x.reshape(N_CORES, LB, 128, 64)  # [c, lb, j, l]
    arr = xc.transpose(0, 3, 1, 2).reshape(N_CORES, 64, 2, 4, 128)
    xt_all = arr.transpose(0, 2, 1, 3, 4).reshape(N_CORES, 128, 512)

    inA = np.zeros((N_CORES, 128, 512), dtype=np.float32)
    inA[:, :64, 0:64] = f2.T
    inA[:, 64:, 64:128] = f2.T
    inA[:, :, 128:256] = f1.T
    inA[:, :, 256:512] = xt_all[:, :, 0:256]
    inB = np.empty((N_CORES, 128, 320), dtype=np.float32)
    inB[:, :, 0:256] = xt_all[:, :, 256:512]
    inB[:, :, 256:320] = bias.reshape(128, 64)[None]

    return [
        {"inpA": np.ascontiguousarray(inA[c]), "inpB": np.ascontiguousarray(inB[c])}
        for c in range(N_CORES)
    ]


def kernel(x, factor1, factor2, bias):
    from concourse.bass_utils import run_bass_kernel_spmd

    if "nc" not in _CACHE:
        _CACHE["nc"] = _build_nc()
    nc = _CACHE["nc"]

    in_maps = _prep_core_inputs(x, factor1, factor2, bias)
    res = run_bass_kernel_spmd(nc, in_maps, core_ids=list(range(N_CORES)))
    kernel.last_results = res

    # device layout: y[i, p*128 + h*64 + k] = out[c*8 + p + 4h, i*64 + k]
    # row order after reshape is r = 2p + h; batch lb = p + 4h -> inv perm
    inv = np.array([0, 2, 4, 6, 1, 3, 5, 7])
    outs = []
    for c in range(N_CORES):
        yc = np.concatenate(
            [res.results[c]["y0"], res.results[c]["y1"]], axis=1
        )  # [128, 512]
        yc = yc.reshape(128, 4, 2, 64).transpose(1, 2, 0, 3).reshape(8, 8192)
        outs.append(yc[inv])
    return np.concatenate(outs, axis=0)


# revision 8
# speedup vs baseline: 1.3333x; 1.2092x over previous
"""KroneckerLinear Trainium2 kernel.

Math: out = x @ kron(f1, f2).T + bias, with x [64, 8192], f1 [128,128],
f2 [64,64], bias [8192].  Kronecker identity:
    out[b].reshape(128, 64) = f1 @ X_b @ f2.T,   X_b = x[b].reshape(128, 64)
so the 8192x8192 weight (256 MB) is never materialized; the kernel is
memory-bound on x in / out (~4 MB total).

Sharding: batch-parallel over the 8 NeuronCores, 8 batch rows per core.

Per-core device program (SPMD, identical on all cores):
  loads: matmul operands in bf16 (halves DMA bytes; PE runs 1 cycle/row),
     bias fp32 (riding in the bf16 tensor via a bitcast view; the final
     add runs in fp32 against the fp32 PSUM).  Two parallel DMAs on the
     two HWDGE rings (sync + scalar), each fully contiguous.
     A = blk | f1t | xt0 | xt1, B = xt2 | xt3 | biasr.
     xt[h*64+l, p*128+j] = x[lb, j*64+l] for local batch lb = p + 4h;
     blk = blkdiag(f2.T, f2.T) so one K=128 matmul computes TWO batches.
  stage 1 (apply f2): matmul p: lhsT = xt_p, rhs = blk ->
     psum_v[p][j, h*64+k] = (X_{p+4h} @ f2.T)[j, k], one PSUM tile per p
     so each PSUM->SBUF cast only waits on its own matmul.  Casts
     (fp32 PSUM -> bf16 v) alternate scalar (ACTIVATE) / vector (DVE);
     gpsimd can't read PSUM on TRN2.
  stage 2 (apply f1): two matmuls lhsT = f1t, rhs = v half [128, 256].
  bias: fused with the PSUM->SBUF move on DVE, fp32.
  stores: two contiguous fp32 DMAs (sync + scalar rings in parallel) of
     the device-natural layout y[i, p*128+h*64+k]; host unpermutes.
"""

import numpy as np

N_CORES = 8
B = 64
LB = B // N_CORES  # 8 local batches per core

_CACHE = {}


def _build_nc():
    import concourse.bass as bass
    import concourse.mybir as mybir
    import concourse.tile as tile
    from concourse import bacc

    fp32 = mybir.dt.float32
    bf16 = mybir.dt.bfloat16

    nc = bacc.Bacc("TRN2", target_bir_lowering=False, debug=False)
    # contiguous per-ring inputs:
    # A: blk 0:128 | f1t 128:256 | xt0 256:384 | xt1 384:512   (bf16)
    # B: xt2 0:128 | xt3 128:256 | biasr-as-bf16 256:384       (bf16)
    inA_d = nc.dram_tensor("inpA", [128, 512], bf16, kind="ExternalInput")
    inB_d = nc.dram_tensor("inpB", [128, 384], bf16, kind="ExternalInput")
    y0_d = nc.dram_tensor("y0", [128, 256], fp32, kind="ExternalOutput")
    y1_d = nc.dram_tensor("y1", [128, 256], fp32, kind="ExternalOutput")

    with tile.TileContext(nc) as tc:
        with (
            tc.tile_pool(name="sb", bufs=1) as sb,
            tc.tile_pool(name="psv", bufs=1, space="PSUM") as psv,
            tc.tile_pool(name="psy", bufs=1, space="PSUM") as psy,
        ):
            sbA = sb.tile([128, 512], bf16)
            sbB = sb.tile([128, 384], bf16)
            v = sb.tile([128, 512], bf16)
            out0 = sb.tile([128, 256], fp32)
            out1 = sb.tile([128, 256], fp32)

            blk = sbA[:, 0:128]
            f1t = sbA[:, 128:256]
            biasr = sbB[:, 256:384].bitcast(fp32)  # [128, 64] fp32

            def xt_slice(p):
                return (
                    sbA[:, 256 + p * 128 : 384 + p * 128]
                    if p < 2
                    else sbB[:, (p - 2) * 128 : (p - 1) * 128]
                )

            # two parallel input DMAs, one per HWDGE ring
            nc.sync.dma_start(out=sbA[:, :], in_=inA_d[:, :])
            nc.scalar.dma_start(out=sbB[:, :], in_=inB_d[:, :])

            b_ap = biasr
            bias_bcast = bass.AP(
                tensor=b_ap.tensor,
                offset=b_ap.offset,
                ap=[b_ap.ap[0], [0, 4], b_ap.ap[1]],
            )

            # stage 1: 4 matmuls, one PSUM tile per slice
            psum_v = []
            for p in range(4):
                pv = psv.tile([128, 128], fp32, tag=f"psum_v{p}")
                psum_v.append(pv)
                nc.tensor.matmul(pv[:], xt_slice(p), blk, start=True, stop=True)
            # PSUM -> SBUF casts (fp32 -> bf16), alternating scalar/vector
            for p in range(4):
                dst = v[:, p * 128 : (p + 1) * 128]
                if p % 2 == 0:
                    nc.scalar.copy(dst, psum_v[p][:])
                else:
                    nc.vector.tensor_copy(dst, psum_v[p][:])

            # stage 2: two N=256 matmuls
            psum_y = []
            for hlf in range(2):
                py = psy.tile([128, 256], fp32, tag=f"psum_y{hlf}")
                psum_y.append(py)
                nc.tensor.matmul(
                    py[:],
                    f1t,
                    v[:, hlf * 256 : (hlf + 1) * 256],
                    start=True,
                    stop=True,
                )

            # bias add fused with PSUM->SBUF move (DVE, fp32)
            for hlf, out_sb in enumerate([out0, out1]):
                o_ap = out_sb[:, :]
                out_g = bass.AP(
                    tensor=o_ap.tensor,
                    offset=o_ap.offset,
                    ap=[o_ap.ap[0], [64, 4], [1, 64]],
                )
                y_ap = psum_y[hlf][:, :]
                y_g = bass.AP(
                    tensor=y_ap.tensor,
                    offset=y_ap.offset,
                    ap=[y_ap.ap[0], [64, 4], [1, 64]],
                )
                nc.vector.tensor_add(out_g, y_g, bias_bcast)

            # two parallel contiguous output DMAs
            nc.sync.dma_start(out=y0_d[:, :], in_=out0[:, :])
            nc.scalar.dma_start(out=y1_d[:, :], in_=out1[:, :])

    nc.compile()
    return nc


def _prep_core_inputs(x, factor1, factor2, bias):
    """Host-side layout prep. Returns list of per-core in_maps."""
    import ml_dtypes

    bf16 = ml_dtypes.bfloat16
    x = np.ascontiguousarray(np.asarray(x, dtype=np.float32))
    f1 = np.asarray(factor1, dtype=np.float32)
    f2 = np.asarray(factor2, dtype=np.float32)
    bias = np.asarray(bias, dtype=np.float32)

    # x -> per-core xt [128, 512]: xt[h*64+l, p*128+j] = x[c*8 + p + 4h, j*64+l]
    xc = x.reshape(N_CORES, LB, 128, 64)  # [c, lb, j, l]
    arr = xc.transpose(0, 3, 1, 2).reshape(N_CORES, 64, 2, 4, 128)
    xt_all = arr.transpose(0, 2, 1, 3, 4).reshape(N_CORES, 128, 512).astype(bf16)

    inA = np.zeros((N_CORES, 128, 512), dtype=bf16)
    f2t = f2.T.astype(bf16)
    inA[:, :64, 0:64] = f2t
    inA[:, 64:, 64:128] = f2t
    inA[:, :, 128:256] = f1.T.astype(bf16)
    inA[:, :, 256:512] = xt_all[:, :, 0:256]
    inB = np.empty((N_CORES, 128, 384), dtype=bf16)
    inB[:, :, 0:256] = xt_all[:, :, 256:512]
    bias_bf = np.ascontiguousarray(bias.reshape(128, 64)).view(bf16)  # [128,128]
    inB[:, :, 256:384] = bias_bf[None]

    return [
        {"inpA": np.ascontiguousarray(inA[c]), "inpB": np.ascontiguousarray(inB[c])}
        for c in range(N_CORES)
    ]


def kernel(x, factor1, factor2, bias):
    from concourse.bass_utils import run_bass_kernel_spmd

    if "nc" not in _CACHE:
        _CACHE["nc"] = _build_nc()
    nc = _CACHE["nc"]

    in_maps = _prep_core_inputs(x, factor1, factor2, bias)
    res = run_bass_kernel_spmd(nc, in_maps, core_ids=list(range(N_CORES)))
    kernel.last_results = res

    # device layout: y[i, p*128 + h*64 + k] = out[c*8 + p + 4h, i*64 + k]
    # row order after reshape is r = 2p + h; batch lb = p + 4h -> inv perm
    inv = np.array([0, 2, 4, 6, 1, 3, 5, 7])
    outs = []
    for c in range(N_CORES):
        yc = np.concatenate(
            [res.results[c]["y0"], res.results[c]["y1"]], axis=1
        )  # [128, 512]
        yc = yc.reshape(128, 4, 2, 64).transpose(1, 2, 0, 3).reshape(8, 8192)
        outs.append(yc[inv])
    return np.concatenate(outs, axis=0)
